# revision 22
# baseline (speedup 1.0000x reference)
"""Trainium2 Bass kernel for batched single-head attention.

Problem: x[8, 4096, 512] fp32, Wq/Wk/Wv[512, 256], bq/bk/bv[256].
  Q = x@Wq + bq ; K = x@Wk + bk ; V = x@Wv + bv
  out = softmax(Q K^T / sqrt(256)) V          -> [8, 4096, 256]

Sharding: data-parallel over batch. 8 batch elements -> 8 NeuronCores,
one full attention per core, no collectives. x is cast to bf16 on the
host (input prep) and transposed on-device via PE matmul-with-identity
(a host-side pre-transpose measured slower end-to-end: the projection
phase then stalls on the serial xT DMA stream instead of overlapping
with the transpose matmuls).

All matmuls run in bf16 with fp32 PSUM accumulation (fp32 matmuls on
TRN2 lower to an FP32HI/FP32LO pass pair AND stream the moving operand
at half rate — measured ~4x slower than bf16). Biases are added in fp32
on the PSUM->SBUF copy; softmax row sums / normalization stay fp32.

Per-core algorithm:
  0. xT = x.T via chunked DMA-transpose (DMA xbar path, zero PE cost).
  1. QT/KT [e, s] = W.T @ xT (weights stationary, N=512 moving), bias
     added on the PSUM->SBUF copy via per-partition activation bias.
  2. V [s, e] natural layout (xT chunks stationary), bias via a rank-1
     (K=1) ones @ bv matmul into the same PSUM group. A ones column is
     appended to V so attn@V also yields softmax row sums for free.
  3. Per q-block of 512: scoresT [k, q] = KT.T @ QT block (PE), exp((.)/16)
     on ACT directly PSUM->SBUF (no max subtraction: scores ~ N(0,1), exp
     is fp32-safe), then out[q, 0:257] += PT_chunk.T @ Vext per k-chunk.
     Scores run 2 k-tiles ahead of attn@V (software pipeline) so the PE
     never waits on the ACT exp latency. Normalize with the fp32 row
     sums (col 256) on the DVE on the way out.
"""

import sys

if "/opt/trn_rl_repo" not in sys.path:
    sys.path.insert(0, "/opt/trn_rl_repo")

import ml_dtypes
import numpy as np

import concourse.bass as bass  # noqa: F401
import concourse.mybir as mybir
import concourse.tile as tile
from concourse import bacc
from concourse.bass_utils import run_bass_kernel_spmd

FP32 = mybir.dt.float32
BF16 = mybir.dt.bfloat16
AF = mybir.ActivationFunctionType

N_CORES = 8
B, S, DIN, D = 8, 4096, 512, 256
P = 128
S_TILES = S // P      # 32 s-tiles
DC = DIN // P         # 4 din chunks
ECH = D // P          # 2 e chunks
QB = 512              # q-block width (columns of scoresT)
N_QB = S // QB        # 8 q-blocks
VE = D + 1            # V columns + ones column = 257
VE_PAD = 260          # padded free extent for the Vext tile
SCALE = 0.0625        # 1/sqrt(256), exact in fp32


def build_program():
    nc = bacc.Bacc(
        "TRN2", target_bir_lowering=False, debug=False, num_devices=N_CORES
    )
    x_d = nc.dram_tensor("x", [S, DIN], BF16, kind="ExternalInput")
    wq_d = nc.dram_tensor("Wq", [DIN, D], BF16, kind="ExternalInput")
    bq_d = nc.dram_tensor("bq", [D], FP32, kind="ExternalInput")
    wk_d = nc.dram_tensor("Wk", [DIN, D], BF16, kind="ExternalInput")
    bk_d = nc.dram_tensor("bk", [D], FP32, kind="ExternalInput")
    wv_d = nc.dram_tensor("Wv", [DIN, D], BF16, kind="ExternalInput")
    bv_d = nc.dram_tensor("bv", [D], BF16, kind="ExternalInput")
    out_d = nc.dram_tensor("out", [S, D], FP32, kind="ExternalOutput")

    with tile.TileContext(nc) as tc:
        with (
            tc.tile_pool(name="const", bufs=1) as constp,
            tc.tile_pool(name="big", bufs=1) as bigp,
        ):
            ones_row = constp.tile([1, P], BF16)
            nc.gpsimd.memset(ones_row[:], 1.0)

            # Weights: [128, 4, 256] with [:, c, :] = W[c*128:(c+1)*128, :]
            # (constants go on the GpSimd DMA queue so the bulk x loads on
            # the Sync queue aren't stuck behind their many descriptors)
            wq_sb = constp.tile([P, DC, D], BF16)
            wk_sb = constp.tile([P, DC, D], BF16)
            wv_sb = constp.tile([P, DC, D], BF16)
            nc.gpsimd.dma_start(wq_sb[:], wq_d.rearrange("(c p) d -> p c d", p=P))
            nc.gpsimd.dma_start(wk_sb[:], wk_d.rearrange("(c p) d -> p c d", p=P))
            nc.gpsimd.dma_start(wv_sb[:], wv_d.rearrange("(c p) d -> p c d", p=P))
            # Per-partition bias layout for QT/KT: [:, c] = b[c*128:(c+1)*128]
            bqT = constp.tile([P, ECH], FP32)
            bkT = constp.tile([P, ECH], FP32)
            nc.gpsimd.dma_start(bqT[:], bq_d.rearrange("(c p) -> p c", p=P))
            nc.gpsimd.dma_start(bkT[:], bk_d.rearrange("(c p) -> p c", p=P))
            # Row layout for the V bias rank-1 update
            bv_row = constp.tile([1, D], BF16)
            nc.gpsimd.dma_start(bv_row[:], bv_d.rearrange("(o d) -> o d", o=1))

            qt = bigp.tile([P, ECH, S], BF16)   # QT: [e-chunk part, ec, s]
            kt = bigp.tile([P, ECH, S], BF16)
            vext = bigp.tile([P, S_TILES, VE_PAD], BF16)  # V + ones col
            nc.gpsimd.memset(vext[:, :, D : D + 1], 1.0)

            # bv broadcast to all partitions (one rank-1 matmul + copy);
            # V tiles then get the bias via a DVE add on the PSUM->SBUF
            # copy instead of a rank-1 matmul per s-tile.
            bv_bc = constp.tile([P, D], BF16)
            with tc.tile_pool(name="bvp", bufs=1, space="PSUM") as bvp:
                psb = bvp.tile([P, D], FP32)
                nc.tensor.matmul(
                    psb[:], ones_row[:], bv_row[:], start=True, stop=True
                )
                nc.vector.tensor_copy(bv_bc[:], psb[:])

            # ---- Phase 1+2 fused: xT via chunked DMA-transpose (the DMA
            # xbar transpose path — separate hardware from the engine
            # ports, so the PE spends zero time transposing), with each
            # s-block's projections (Q/K/V) emitted right after its
            # 4 transposed chunks are in flight. ----
            with tc.tile_pool(name="xTpool", bufs=1) as xtp:
                xt = xtp.tile([P, DC, S], BF16)  # xT: [din-chunk part, dc, s]
                with (
                    tc.tile_pool(name="pjq", bufs=3, space="PSUM") as pjq,
                    tc.tile_pool(name="pjv", bufs=2, space="PSUM") as pjv,
                ):
                    for sb in range(N_QB):
                        for dc in range(DC):
                            nc.sync.dma_start_transpose(
                                xt[:, dc, sb * QB : (sb + 1) * QB],
                                x_d[
                                    sb * QB : (sb + 1) * QB,
                                    dc * P : (dc + 1) * P,
                                ],
                            )
                        for w_sb, bT, dst in (
                            (wq_sb, bqT, qt),
                            (wk_sb, bkT, kt),
                        ):
                            for ec in range(ECH):
                                ps = pjq.tile([P, QB], FP32)
                                for dc in range(DC):
                                    nc.tensor.matmul(
                                        ps[:],
                                        w_sb[:, dc, ec * P : (ec + 1) * P],
                                        xt[:, dc, sb * QB : (sb + 1) * QB],
                                        start=(dc == 0),
                                        stop=(dc == DC - 1),
                                    )
                                nc.scalar.activation(
                                    dst[:, ec, sb * QB : (sb + 1) * QB],
                                    ps[:],
                                    AF.Identity,
                                    bias=bT[:, ec : ec + 1],
                                )
                        for stv in range(sb * 4, sb * 4 + 4):
                            psv = pjv.tile([P, D], FP32)
                            for dc in range(DC):
                                nc.tensor.matmul(
                                    psv[:],
                                    xt[:, dc, stv * P : (stv + 1) * P],
                                    wv_sb[:, dc, :],
                                    start=(dc == 0),
                                    stop=(dc == DC - 1),
                                )
                            nc.vector.tensor_add(
                                vext[:, stv, 0:D], psv[:], bv_bc[:]
                            )

            # ---- Phase 3: attention (software-pipelined: scores run
            # LOOKAHEAD k-tiles ahead of attn@V so the PE never waits on
            # the ACT exp latency) ----
            LOOKAHEAD = 3
            NSTEPS = N_QB * S_TILES
            with (
                tc.tile_pool(name="ptp", bufs=5) as ptp,
                tc.tile_pool(name="accp", bufs=5, space="PSUM") as accp,
                tc.tile_pool(name="scp", bufs=3, space="PSUM") as scp,
                tc.tile_pool(name="outp", bufs=4) as outp,
                tc.tile_pool(name="nrmp", bufs=4) as nrmp,
            ):
                accs = {}
                ptts = {}
                # one flat loop over (q-block, k-tile) so the scores
                # lookahead also spans q-block boundaries
                for step in range(NSTEPS + LOOKAHEAD):
                    if step < NSTEPS:
                        qb, kt_i = divmod(step, S_TILES)
                        if kt_i == 0:
                            accs[qb] = [
                                accp.tile([P, VE], FP32, name="acc", tag="acc")
                                for _ in range(QB // P)
                            ]
                        pss = scp.tile([P, QB], FP32)
                        for ec in range(ECH):
                            nc.tensor.matmul(
                                pss[:],
                                kt[:, ec, kt_i * P : (kt_i + 1) * P],
                                qt[:, ec, qb * QB : (qb + 1) * QB],
                                start=(ec == 0),
                                stop=(ec == ECH - 1),
                            )
                        ptt = ptp.tile([P, QB], BF16)
                        nc.scalar.activation(
                            ptt[:], pss[:], AF.Exp, scale=SCALE
                        )
                        ptts[step] = ptt
                    av = step - LOOKAHEAD
                    if av >= 0:
                        qb2, kt2 = divmod(av, S_TILES)
                        pav = ptts.pop(av)
                        for j in range(QB // P):
                            nc.tensor.matmul(
                                accs[qb2][j][:],
                                pav[:, j * P : (j + 1) * P],
                                vext[:, kt2, 0:VE],
                                start=(kt2 == 0),
                                stop=(kt2 == S_TILES - 1),
                            )
                        if kt2 == S_TILES - 1:
                            for j in range(QB // P):
                                rc = nrmp.tile([P, 1], FP32)
                                nc.vector.reciprocal(
                                    rc[:], accs[qb2][j][:, D : D + 1]
                                )
                                ot = outp.tile([P, D], FP32)
                                nc.vector.tensor_scalar_mul(
                                    ot[:], accs[qb2][j][:, 0:D], rc[:]
                                )
                                row = (qb2 * (QB // P) + j) * P
                                nc.sync.dma_start(
                                    out_d[row : row + P, :], ot[:]
                                )
                            del accs[qb2]

    nc.compile()
    return nc


_NC_CACHE = []


def _get_nc():
    if not _NC_CACHE:
        _NC_CACHE.append(build_program())
    return _NC_CACHE[0]


def kernel(**inputs) -> np.ndarray:
    BF = ml_dtypes.bfloat16
    x = np.ascontiguousarray(np.asarray(inputs["x"]).astype(BF))
    w = {}
    for k in ("Wq", "Wk", "Wv", "bv"):
        w[k] = np.ascontiguousarray(np.asarray(inputs[k]).astype(BF))
    for k in ("bq", "bk"):
        w[k] = np.ascontiguousarray(np.asarray(inputs[k]).astype(np.float32))
    nc = _get_nc()
    in_maps = [{"x": x[b], **w} for b in range(B)]
    res = run_bass_kernel_spmd(nc, in_maps, list(range(N_CORES)))
    return np.stack([res.results[b]["out"] for b in range(B)], axis=0)


# revision 23
# speedup vs baseline: 1.0399x; 1.0399x over previous
"""Trainium2 Bass kernel for batched single-head attention.

Problem: x[8, 4096, 512] fp32, Wq/Wk/Wv[512, 256], bq/bk/bv[256].
  Q = x@Wq + bq ; K = x@Wk + bk ; V = x@Wv + bv
  out = softmax(Q K^T / sqrt(256)) V          -> [8, 4096, 256]

Sharding: data-parallel over batch. 8 batch elements -> 8 NeuronCores,
one full attention per core, no collectives. x is cast to bf16 on the
host (input prep) and transposed on-device via PE matmul-with-identity
(a host-side pre-transpose measured slower end-to-end: the projection
phase then stalls on the serial xT DMA stream instead of overlapping
with the transpose matmuls).

All matmuls run in bf16 with fp32 PSUM accumulation (fp32 matmuls on
TRN2 lower to an FP32HI/FP32LO pass pair AND stream the moving operand
at half rate — measured ~4x slower than bf16). Biases are added in fp32
on the PSUM->SBUF copy; softmax row sums / normalization stay fp32.

Per-core algorithm:
  0. xT = x.T via PE matmul-with-identity; 4 chunks per full PSUM bank,
     one strided cast PSUM->SBUF per s-tile, alternating DVE/ACT.
  1. QT/KT [e, s] = W.T @ xT (weights stationary, N=512 moving), bias
     added on the PSUM->SBUF copy via per-partition activation bias.
  2. V [s, e] natural layout (xT chunks stationary), bias via a rank-1
     (K=1) ones @ bv matmul into the same PSUM group. A ones column is
     appended to V so attn@V also yields softmax row sums for free.
  3. Per q-block of 512: scoresT [k, q] = KT.T @ QT block (PE), exp((.)/16)
     on ACT directly PSUM->SBUF (no max subtraction: scores ~ N(0,1), exp
     is fp32-safe), then out[q, 0:257] += PT_chunk.T @ Vext per k-chunk.
     Scores run 2 k-tiles ahead of attn@V (software pipeline) so the PE
     never waits on the ACT exp latency. Normalize with the fp32 row
     sums (col 256) on the DVE on the way out.
"""

import sys

if "/opt/trn_rl_repo" not in sys.path:
    sys.path.insert(0, "/opt/trn_rl_repo")

import ml_dtypes
import numpy as np

import concourse.bass as bass  # noqa: F401
import concourse.mybir as mybir
import concourse.tile as tile
from concourse import bacc
from concourse.bass_utils import run_bass_kernel_spmd
from concourse.masks import make_identity

FP32 = mybir.dt.float32
BF16 = mybir.dt.bfloat16
AF = mybir.ActivationFunctionType

N_CORES = 8
B, S, DIN, D = 8, 4096, 512, 256
P = 128
S_TILES = S // P      # 32 s-tiles
DC = DIN // P         # 4 din chunks
ECH = D // P          # 2 e chunks
QB = 512              # q-block width (columns of scoresT)
N_QB = S // QB        # 8 q-blocks
VE = D + 1            # V columns + ones column = 257
VE_PAD = 260          # padded free extent for the Vext tile
SCALE = 0.0625        # 1/sqrt(256), exact in fp32


def build_program():
    nc = bacc.Bacc(
        "TRN2", target_bir_lowering=False, debug=False, num_devices=N_CORES
    )
    x_d = nc.dram_tensor("x", [S, DIN], BF16, kind="ExternalInput")
    wq_d = nc.dram_tensor("Wq", [DIN, D], BF16, kind="ExternalInput")
    bq_d = nc.dram_tensor("bq", [D], FP32, kind="ExternalInput")
    wk_d = nc.dram_tensor("Wk", [DIN, D], BF16, kind="ExternalInput")
    bk_d = nc.dram_tensor("bk", [D], FP32, kind="ExternalInput")
    wv_d = nc.dram_tensor("Wv", [DIN, D], BF16, kind="ExternalInput")
    bv_d = nc.dram_tensor("bv", [D], BF16, kind="ExternalInput")
    out_d = nc.dram_tensor("out", [S, D], FP32, kind="ExternalOutput")

    with tile.TileContext(nc) as tc:
        with (
            tc.tile_pool(name="const", bufs=1) as constp,
            tc.tile_pool(name="big", bufs=1) as bigp,
        ):
            ident = constp.tile([P, P], BF16)
            make_identity(nc, ident[:])
            ones_row = constp.tile([1, P], BF16)
            nc.gpsimd.memset(ones_row[:], 1.0)

            # Weights: [128, 4, 256] with [:, c, :] = W[c*128:(c+1)*128, :]
            # (constants go on the GpSimd DMA queue so the bulk x loads on
            # the Sync queue aren't stuck behind their many descriptors)
            wq_sb = constp.tile([P, DC, D], BF16)
            wk_sb = constp.tile([P, DC, D], BF16)
            wv_sb = constp.tile([P, DC, D], BF16)
            nc.gpsimd.dma_start(wq_sb[:], wq_d.rearrange("(c p) d -> p c d", p=P))
            nc.gpsimd.dma_start(wk_sb[:], wk_d.rearrange("(c p) d -> p c d", p=P))
            nc.gpsimd.dma_start(wv_sb[:], wv_d.rearrange("(c p) d -> p c d", p=P))
            # Per-partition bias layout for QT/KT: [:, c] = b[c*128:(c+1)*128]
            bqT = constp.tile([P, ECH], FP32)
            bkT = constp.tile([P, ECH], FP32)
            nc.gpsimd.dma_start(bqT[:], bq_d.rearrange("(c p) -> p c", p=P))
            nc.gpsimd.dma_start(bkT[:], bk_d.rearrange("(c p) -> p c", p=P))
            # Row layout for the V bias rank-1 update
            bv_row = constp.tile([1, D], BF16)
            nc.gpsimd.dma_start(bv_row[:], bv_d.rearrange("(o d) -> o d", o=1))

            qt = bigp.tile([P, ECH, S], BF16)   # QT: [e-chunk part, ec, s]
            kt = bigp.tile([P, ECH, S], BF16)
            vext = bigp.tile([P, S_TILES, VE_PAD], BF16)  # V + ones col
            nc.gpsimd.memset(vext[:, :, D : D + 1], 1.0)

            # bv broadcast to all partitions (one rank-1 matmul + copy);
            # V tiles then get the bias via a DVE add on the PSUM->SBUF
            # copy instead of a rank-1 matmul per s-tile.
            bv_bc = constp.tile([P, D], BF16)
            with tc.tile_pool(name="bvp", bufs=1, space="PSUM") as bvp:
                psb = bvp.tile([P, D], FP32)
                nc.tensor.matmul(
                    psb[:], ones_row[:], bv_row[:], start=True, stop=True
                )
                nc.vector.tensor_copy(bv_bc[:], psb[:])

            # ---- Phase 1+2 fused: transpose x -> xT, with each s-block's
            # projections (Q/K/V) emitted right after its 4 transposes.
            # Interleaving the tiny N=128 transpose matmuls with N=512
            # projection streams lets the PE's LDWEIGHTS pull-ahead hide
            # the transpose weight loads under the projection streams. ----
            with tc.tile_pool(name="xTpool", bufs=1) as xtp:
                xt = xtp.tile([P, DC, S], BF16)  # xT: [din-chunk part, dc, s]
                with (
                    tc.tile_pool(name="xload", bufs=3) as xlp,
                    tc.tile_pool(name="tps", bufs=3, space="PSUM") as tpsp,
                    tc.tile_pool(name="pjq", bufs=3, space="PSUM") as pjq,
                    tc.tile_pool(name="pjv", bufs=2, space="PSUM") as pjv,
                ):
                    for st in range(S_TILES):
                        xtile = xlp.tile([P, DIN], BF16)
                        nc.sync.dma_start(
                            xtile[:], x_d[st * P : (st + 1) * P, :]
                        )
                        # 4 transposed chunks into one full PSUM bank,
                        # then a single strided cast to SBUF.
                        pst = tpsp.tile([P, DIN], FP32)
                        for dc in range(DC):
                            nc.tensor.matmul(
                                pst[:, dc * P : (dc + 1) * P],
                                xtile[:, dc * P : (dc + 1) * P],
                                ident[:],
                                start=True,
                                stop=True,
                            )
                        src = pst[:].rearrange("p (c f) -> p c f", c=DC)
                        dstv = xt[:, :, st * P : (st + 1) * P]
                        if st % 2 == 0:
                            nc.vector.tensor_copy(dstv, src)
                        else:
                            nc.scalar.copy(dstv, src)

                        if st % 4 != 3:
                            continue
                        sb = st // 4  # this s-block's xT is now complete
                        for w_sb, bT, dst in (
                            (wq_sb, bqT, qt),
                            (wk_sb, bkT, kt),
                        ):
                            for ec in range(ECH):
                                ps = pjq.tile([P, QB], FP32)
                                for dc in range(DC):
                                    nc.tensor.matmul(
                                        ps[:],
                                        w_sb[:, dc, ec * P : (ec + 1) * P],
                                        xt[:, dc, sb * QB : (sb + 1) * QB],
                                        start=(dc == 0),
                                        stop=(dc == DC - 1),
                                    )
                                nc.scalar.activation(
                                    dst[:, ec, sb * QB : (sb + 1) * QB],
                                    ps[:],
                                    AF.Identity,
                                    bias=bT[:, ec : ec + 1],
                                )
                        for stv in range(sb * 4, sb * 4 + 4):
                            psv = pjv.tile([P, D], FP32)
                            for dc in range(DC):
                                nc.tensor.matmul(
                                    psv[:],
                                    xt[:, dc, stv * P : (stv + 1) * P],
                                    wv_sb[:, dc, :],
                                    start=(dc == 0),
                                    stop=(dc == DC - 1),
                                )
                            nc.vector.tensor_add(
                                vext[:, stv, 0:D], psv[:], bv_bc[:]
                            )

            # ---- Phase 3: attention (software-pipelined: scores run
            # LOOKAHEAD k-tiles ahead of attn@V so the PE never waits on
            # the ACT exp latency) ----
            LOOKAHEAD = 3
            NSTEPS = N_QB * S_TILES
            with (
                tc.tile_pool(name="ptp", bufs=5) as ptp,
                tc.tile_pool(name="accp", bufs=5, space="PSUM") as accp,
                tc.tile_pool(name="scp", bufs=3, space="PSUM") as scp,
                tc.tile_pool(name="outp", bufs=4) as outp,
                tc.tile_pool(name="nrmp", bufs=4) as nrmp,
            ):
                accs = {}
                ptts = {}
                # one flat loop over (q-block, k-tile) so the scores
                # lookahead also spans q-block boundaries
                for step in range(NSTEPS + LOOKAHEAD):
                    if step < NSTEPS:
                        qb, kt_i = divmod(step, S_TILES)
                        if kt_i == 0:
                            accs[qb] = [
                                accp.tile([P, VE], FP32, name="acc", tag="acc")
                                for _ in range(QB // P)
                            ]
                        pss = scp.tile([P, QB], FP32)
                        for ec in range(ECH):
                            nc.tensor.matmul(
                                pss[:],
                                kt[:, ec, kt_i * P : (kt_i + 1) * P],
                                qt[:, ec, qb * QB : (qb + 1) * QB],
                                start=(ec == 0),
                                stop=(ec == ECH - 1),
                            )
                        ptt = ptp.tile([P, QB], BF16)
                        nc.scalar.activation(
                            ptt[:], pss[:], AF.Exp, scale=SCALE
                        )
                        ptts[step] = ptt
                    av = step - LOOKAHEAD
                    if av >= 0:
                        qb2, kt2 = divmod(av, S_TILES)
                        pav = ptts.pop(av)
                        for j in range(QB // P):
                            nc.tensor.matmul(
                                accs[qb2][j][:],
                                pav[:, j * P : (j + 1) * P],
                                vext[:, kt2, 0:VE],
                                start=(kt2 == 0),
                                stop=(kt2 == S_TILES - 1),
                            )
                        if kt2 == S_TILES - 1:
                            for j in range(QB // P):
                                rc = nrmp.tile([P, 1], FP32)
                                nc.vector.reciprocal(
                                    rc[:], accs[qb2][j][:, D : D + 1]
                                )
                                ot = outp.tile([P, D], FP32)
                                nc.vector.tensor_scalar_mul(
                                    ot[:], accs[qb2][j][:, 0:D], rc[:]
                                )
                                row = (qb2 * (QB // P) + j) * P
                                nc.sync.dma_start(
                                    out_d[row : row + P, :], ot[:]
                                )
                            del accs[qb2]

    nc.compile()
    return nc


_NC_CACHE = []


def _get_nc():
    if not _NC_CACHE:
        _NC_CACHE.append(build_program())
    return _NC_CACHE[0]


def kernel(**inputs) -> np.ndarray:
    BF = ml_dtypes.bfloat16
    x = np.ascontiguousarray(np.asarray(inputs["x"]).astype(BF))
    w = {}
    for k in ("Wq", "Wk", "Wv", "bv"):
        w[k] = np.ascontiguousarray(np.asarray(inputs[k]).astype(BF))
    for k in ("bq", "bk"):
        w[k] = np.ascontiguousarray(np.asarray(inputs[k]).astype(np.float32))
    nc = _get_nc()
    in_maps = [{"x": x[b], **w} for b in range(B)]
    res = run_bass_kernel_spmd(nc, in_maps, list(range(N_CORES)))
    return np.stack([res.results[b]["out"] for b in range(B)], axis=0)


# revision 24
# speedup vs baseline: 1.0658x; 1.0249x over previous
"""Trainium2 Bass kernel for batched single-head attention.

Problem: x[8, 4096, 512] fp32, Wq/Wk/Wv[512, 256], bq/bk/bv[256].
  Q = x@Wq + bq ; K = x@Wk + bk ; V = x@Wv + bv
  out = softmax(Q K^T / sqrt(256)) V          -> [8, 4096, 256]

Sharding: data-parallel over batch. 8 batch elements -> 8 NeuronCores,
one full attention per core, no collectives. x is cast to bf16 on the
host (input prep) and transposed on-device via PE matmul-with-identity
(a host-side pre-transpose measured slower end-to-end: the projection
phase then stalls on the serial xT DMA stream instead of overlapping
with the transpose matmuls).

All matmuls run in bf16 with fp32 PSUM accumulation (fp32 matmuls on
TRN2 lower to an FP32HI/FP32LO pass pair AND stream the moving operand
at half rate — measured ~4x slower than bf16). Biases are added in fp32
on the PSUM->SBUF copy; softmax row sums / normalization stay fp32.

Per-core algorithm:
  0. xT = x.T via PE matmul-with-identity; 4 chunks per full PSUM bank,
     one strided cast PSUM->SBUF per s-tile, alternating DVE/ACT.
  1. QT/KT [e, s] = W.T @ xT (weights stationary, N=512 moving), bias
     added on the PSUM->SBUF copy via per-partition activation bias.
  2. V [s, e] natural layout (xT chunks stationary), bias via a rank-1
     (K=1) ones @ bv matmul into the same PSUM group. A ones column is
     appended to V so attn@V also yields softmax row sums for free.
  3. Per q-block of 512: scoresT [k, q] = KT.T @ QT block (PE), exp((.)/16)
     on ACT directly PSUM->SBUF (no max subtraction: scores ~ N(0,1), exp
     is fp32-safe), then out[q, 0:257] += PT_chunk.T @ Vext per k-chunk.
     Scores run 2 k-tiles ahead of attn@V (software pipeline) so the PE
     never waits on the ACT exp latency. Normalize with the fp32 row
     sums (col 256) on the DVE on the way out.
"""

import sys

if "/opt/trn_rl_repo" not in sys.path:
    sys.path.insert(0, "/opt/trn_rl_repo")

import ml_dtypes
import numpy as np

import concourse.bass as bass  # noqa: F401
import concourse.mybir as mybir
import concourse.tile as tile
from concourse import bacc
from concourse.bass_utils import run_bass_kernel_spmd
from concourse.masks import make_identity

FP32 = mybir.dt.float32
BF16 = mybir.dt.bfloat16
AF = mybir.ActivationFunctionType

N_CORES = 8
B, S, DIN, D = 8, 4096, 512, 256
P = 128
S_TILES = S // P      # 32 s-tiles
DC = DIN // P         # 4 din chunks
ECH = D // P          # 2 e chunks
QB = 512              # q-block width (columns of scoresT)
N_QB = S // QB        # 8 q-blocks
VE = D + 1            # V columns + ones column = 257
VE_PAD = 260          # padded free extent for the Vext tile
SCALE = 0.0625        # 1/sqrt(256), exact in fp32


def build_program():
    nc = bacc.Bacc(
        "TRN2", target_bir_lowering=False, debug=False, num_devices=N_CORES
    )
    x_d = nc.dram_tensor("x", [S, DIN], BF16, kind="ExternalInput")
    wq_d = nc.dram_tensor("Wq", [DIN, D], BF16, kind="ExternalInput")
    bq_d = nc.dram_tensor("bq", [D], FP32, kind="ExternalInput")
    wk_d = nc.dram_tensor("Wk", [DIN, D], BF16, kind="ExternalInput")
    bk_d = nc.dram_tensor("bk", [D], FP32, kind="ExternalInput")
    wv_d = nc.dram_tensor("Wv", [DIN, D], BF16, kind="ExternalInput")
    bv_d = nc.dram_tensor("bv", [D], BF16, kind="ExternalInput")
    out_d = nc.dram_tensor("out", [S, D], FP32, kind="ExternalOutput")

    with tile.TileContext(nc) as tc:
        with (
            tc.tile_pool(name="const", bufs=1) as constp,
            tc.tile_pool(name="big", bufs=1) as bigp,
        ):
            ident = constp.tile([P, P], BF16)
            make_identity(nc, ident[:])
            ones_row = constp.tile([1, P], BF16)
            nc.gpsimd.memset(ones_row[:], 1.0)

            # Weights: [128, 4, 256] with [:, c, :] = W[c*128:(c+1)*128, :]
            # (constants go on the GpSimd DMA queue so the bulk x loads on
            # the Sync queue aren't stuck behind their many descriptors)
            wq_sb = constp.tile([P, DC, D], BF16)
            wk_sb = constp.tile([P, DC, D], BF16)
            wv_sb = constp.tile([P, DC, D], BF16)
            nc.gpsimd.dma_start(wq_sb[:], wq_d.rearrange("(c p) d -> p c d", p=P))
            nc.gpsimd.dma_start(wk_sb[:], wk_d.rearrange("(c p) d -> p c d", p=P))
            nc.gpsimd.dma_start(wv_sb[:], wv_d.rearrange("(c p) d -> p c d", p=P))
            # Per-partition bias layout for QT/KT: [:, c] = b[c*128:(c+1)*128]
            bqT = constp.tile([P, ECH], FP32)
            bkT = constp.tile([P, ECH], FP32)
            nc.gpsimd.dma_start(bqT[:], bq_d.rearrange("(c p) -> p c", p=P))
            nc.gpsimd.dma_start(bkT[:], bk_d.rearrange("(c p) -> p c", p=P))
            # Row layout for the V bias rank-1 update
            bv_row = constp.tile([1, D], BF16)
            nc.gpsimd.dma_start(bv_row[:], bv_d.rearrange("(o d) -> o d", o=1))

            qt = bigp.tile([P, ECH, S], BF16)   # QT: [e-chunk part, ec, s]
            kt = bigp.tile([P, ECH, S], BF16)
            vext = bigp.tile([P, S_TILES, VE_PAD], BF16)  # V + ones col
            nc.gpsimd.memset(vext[:, :, D : D + 1], 1.0)

            # bv broadcast to all partitions (one rank-1 matmul + copy);
            # V tiles then get the bias via a DVE add on the PSUM->SBUF
            # copy instead of a rank-1 matmul per s-tile.
            bv_bc = constp.tile([P, D], BF16)
            with tc.tile_pool(name="bvp", bufs=1, space="PSUM") as bvp:
                psb = bvp.tile([P, D], FP32)
                nc.tensor.matmul(
                    psb[:], ones_row[:], bv_row[:], start=True, stop=True
                )
                nc.vector.tensor_copy(bv_bc[:], psb[:])

            # ---- Phase 1+2 fused: transpose x -> xT, with each s-block's
            # projections (Q/K/V) emitted right after its 4 transposes.
            # Interleaving the tiny N=128 transpose matmuls with N=512
            # projection streams lets the PE's LDWEIGHTS pull-ahead hide
            # the transpose weight loads under the projection streams. ----
            with tc.tile_pool(name="xTpool", bufs=1) as xtp:
                xt = xtp.tile([P, DC, S], BF16)  # xT: [din-chunk part, dc, s]
                with (
                    tc.tile_pool(name="xload", bufs=8) as xlp,
                    tc.tile_pool(name="tps", bufs=3, space="PSUM") as tpsp,
                    tc.tile_pool(name="pjq", bufs=3, space="PSUM") as pjq,
                    tc.tile_pool(name="pjv", bufs=2, space="PSUM") as pjv,
                ):
                    xtiles, psts = {}, {}

                    def emit_t_dma(st):
                        xtile = xlp.tile([P, DIN], BF16, name="xtile")
                        nc.sync.dma_start(
                            xtile[:], x_d[st * P : (st + 1) * P, :]
                        )
                        xtiles[st] = xtile

                    def emit_t_mm(st, c):
                        # one transposed [128,128] chunk; 4 chunks fill one
                        # PSUM bank, then a single strided cast to SBUF
                        if c == 0:
                            psts[st] = tpsp.tile([P, DIN], FP32, name="pst")
                        nc.tensor.matmul(
                            psts[st][:, c * P : (c + 1) * P],
                            xtiles[st][:, c * P : (c + 1) * P],
                            ident[:],
                            start=True,
                            stop=True,
                        )
                        if c == DC - 1:
                            src = psts.pop(st)[:].rearrange(
                                "p (c f) -> p c f", c=DC
                            )
                            dstv = xt[:, :, st * P : (st + 1) * P]
                            if st % 2 == 0:
                                nc.vector.tensor_copy(dstv, src)
                            else:
                                nc.scalar.copy(dstv, src)
                            xtiles.pop(st)

                    # group 0 has nothing to hide under (pure DMA wait)
                    for st in range(4):
                        emit_t_dma(st)
                    for st in range(4):
                        for c in range(DC):
                            emit_t_mm(st, c)

                    for sb in range(N_QB):
                        # next group's transposes: 16 tiny matmuls, paired
                        # 1:1 with this block's 16 N=512 Q/K matmuls so
                        # their weight loads hide under the streams
                        tmms = []
                        if sb + 1 < N_QB:
                            for st in range(4 * (sb + 1), 4 * (sb + 1) + 4):
                                emit_t_dma(st)
                                tmms += [(st, c) for c in range(DC)]
                        ti = 0
                        for w_sb, bT, dst in (
                            (wq_sb, bqT, qt),
                            (wk_sb, bkT, kt),
                        ):
                            for ec in range(ECH):
                                ps = pjq.tile([P, QB], FP32)
                                for dc in range(DC):
                                    nc.tensor.matmul(
                                        ps[:],
                                        w_sb[:, dc, ec * P : (ec + 1) * P],
                                        xt[:, dc, sb * QB : (sb + 1) * QB],
                                        start=(dc == 0),
                                        stop=(dc == DC - 1),
                                    )
                                    if ti < len(tmms):
                                        emit_t_mm(*tmms[ti])
                                        ti += 1
                                nc.scalar.activation(
                                    dst[:, ec, sb * QB : (sb + 1) * QB],
                                    ps[:],
                                    AF.Identity,
                                    bias=bT[:, ec : ec + 1],
                                )
                        for stv in range(sb * 4, sb * 4 + 4):
                            psv = pjv.tile([P, D], FP32)
                            for dc in range(DC):
                                nc.tensor.matmul(
                                    psv[:],
                                    xt[:, dc, stv * P : (stv + 1) * P],
                                    wv_sb[:, dc, :],
                                    start=(dc == 0),
                                    stop=(dc == DC - 1),
                                )
                            nc.vector.tensor_add(
                                vext[:, stv, 0:D], psv[:], bv_bc[:]
                            )

            # ---- Phase 3: attention (software-pipelined: scores run
            # LOOKAHEAD k-tiles ahead of attn@V so the PE never waits on
            # the ACT exp latency) ----
            LOOKAHEAD = 3
            NSTEPS = N_QB * S_TILES
            with (
                tc.tile_pool(name="ptp", bufs=5) as ptp,
                tc.tile_pool(name="accp", bufs=5, space="PSUM") as accp,
                tc.tile_pool(name="scp", bufs=3, space="PSUM") as scp,
                tc.tile_pool(name="outp", bufs=4) as outp,
                tc.tile_pool(name="nrmp", bufs=4) as nrmp,
            ):
                accs = {}
                ptts = {}
                # one flat loop over (q-block, k-tile) so the scores
                # lookahead also spans q-block boundaries
                for step in range(NSTEPS + LOOKAHEAD):
                    if step < NSTEPS:
                        qb, kt_i = divmod(step, S_TILES)
                        if kt_i == 0:
                            accs[qb] = [
                                accp.tile([P, VE], FP32, name="acc", tag="acc")
                                for _ in range(QB // P)
                            ]
                        pss = scp.tile([P, QB], FP32)
                        for ec in range(ECH):
                            nc.tensor.matmul(
                                pss[:],
                                kt[:, ec, kt_i * P : (kt_i + 1) * P],
                                qt[:, ec, qb * QB : (qb + 1) * QB],
                                start=(ec == 0),
                                stop=(ec == ECH - 1),
                            )
                        ptt = ptp.tile([P, QB], BF16)
                        nc.scalar.activation(
                            ptt[:], pss[:], AF.Exp, scale=SCALE
                        )
                        ptts[step] = ptt
                    av = step - LOOKAHEAD
                    if av >= 0:
                        qb2, kt2 = divmod(av, S_TILES)
                        pav = ptts.pop(av)
                        for j in range(QB // P):
                            nc.tensor.matmul(
                                accs[qb2][j][:],
                                pav[:, j * P : (j + 1) * P],
                                vext[:, kt2, 0:VE],
                                start=(kt2 == 0),
                                stop=(kt2 == S_TILES - 1),
                            )
                        if kt2 == S_TILES - 1:
                            for j in range(QB // P):
                                rc = nrmp.tile([P, 1], FP32)
                                nc.vector.reciprocal(
                                    rc[:], accs[qb2][j][:, D : D + 1]
                                )
                                ot = outp.tile([P, D], FP32)
                                nc.vector.tensor_scalar_mul(
                                    ot[:], accs[qb2][j][:, 0:D], rc[:]
                                )
                                row = (qb2 * (QB // P) + j) * P
                                nc.sync.dma_start(
                                    out_d[row : row + P, :], ot[:]
                                )
                            del accs[qb2]

    nc.compile()
    return nc


_NC_CACHE = []


def _get_nc():
    if not _NC_CACHE:
        _NC_CACHE.append(build_program())
    return _NC_CACHE[0]


def kernel(**inputs) -> np.ndarray:
    BF = ml_dtypes.bfloat16
    x = np.ascontiguousarray(np.asarray(inputs["x"]).astype(BF))
    w = {}
    for k in ("Wq", "Wk", "Wv", "bv"):
        w[k] = np.ascontiguousarray(np.asarray(inputs[k]).astype(BF))
    for k in ("bq", "bk"):
        w[k] = np.ascontiguousarray(np.asarray(inputs[k]).astype(np.float32))
    nc = _get_nc()
    in_maps = [{"x": x[b], **w} for b in range(B)]
    res = run_bass_kernel_spmd(nc, in_maps, list(range(N_CORES)))
    return np.stack([res.results[b]["out"] for b in range(B)], axis=0)


# revision 26
# speedup vs baseline: 1.0767x; 1.0103x over previous
"""Trainium2 Bass kernel for batched single-head attention.

Problem: x[8, 4096, 512] fp32, Wq/Wk/Wv[512, 256], bq/bk/bv[256].
  Q = x@Wq + bq ; K = x@Wk + bk ; V = x@Wv + bv
  out = softmax(Q K^T / sqrt(256)) V          -> [8, 4096, 256]

Sharding: data-parallel over batch. 8 batch elements -> 8 NeuronCores,
one full attention per core, no collectives. x is cast to bf16 on the
host (input prep) and transposed on-device via PE matmul-with-identity
(a host-side pre-transpose measured slower end-to-end: the projection
phase then stalls on the serial xT DMA stream instead of overlapping
with the transpose matmuls).

All matmuls run in bf16 with fp32 PSUM accumulation (fp32 matmuls on
TRN2 lower to an FP32HI/FP32LO pass pair AND stream the moving operand
at half rate — measured ~4x slower than bf16). Biases are added in fp32
on the PSUM->SBUF copy; softmax row sums / normalization stay fp32.

Per-core algorithm:
  0. xT = x.T via PE matmul-with-identity; 4 chunks per full PSUM bank,
     one strided cast PSUM->SBUF per s-tile, alternating DVE/ACT.
  1. QT/KT [e, s] = W.T @ xT (weights stationary, N=512 moving), bias
     added on the PSUM->SBUF copy via per-partition activation bias.
  2. V [s, e] natural layout (xT chunks stationary), bias via a rank-1
     (K=1) ones @ bv matmul into the same PSUM group. A ones column is
     appended to V so attn@V also yields softmax row sums for free.
  3. Per q-block of 512: scoresT [k, q] = KT.T @ QT block (PE), exp((.)/16)
     on ACT directly PSUM->SBUF (no max subtraction: scores ~ N(0,1), exp
     is fp32-safe), then out[q, 0:257] += PT_chunk.T @ Vext per k-chunk.
     Scores run 2 k-tiles ahead of attn@V (software pipeline) so the PE
     never waits on the ACT exp latency. Normalize with the fp32 row
     sums (col 256) on the DVE on the way out.
"""

import sys

if "/opt/trn_rl_repo" not in sys.path:
    sys.path.insert(0, "/opt/trn_rl_repo")

import ml_dtypes
import numpy as np

import concourse.bass as bass  # noqa: F401
import concourse.mybir as mybir
import concourse.tile as tile
from concourse import bacc
from concourse.bass_utils import run_bass_kernel_spmd
from concourse.masks import make_identity

FP32 = mybir.dt.float32
BF16 = mybir.dt.bfloat16
AF = mybir.ActivationFunctionType

N_CORES = 8
B, S, DIN, D = 8, 4096, 512, 256
P = 128
S_TILES = S // P      # 32 s-tiles
DC = DIN // P         # 4 din chunks
ECH = D // P          # 2 e chunks
QB = 512              # q-block width (columns of scoresT)
N_QB = S // QB        # 8 q-blocks
VE = D + 1            # V columns + ones column = 257
VE_PAD = 260          # padded free extent for the Vext tile
SCALE = 0.0625        # 1/sqrt(256), exact in fp32


def build_program():
    nc = bacc.Bacc(
        "TRN2", target_bir_lowering=False, debug=False, num_devices=N_CORES
    )
    x_d = nc.dram_tensor("x", [S, DIN], BF16, kind="ExternalInput")
    wq_d = nc.dram_tensor("Wq", [DIN, D], BF16, kind="ExternalInput")
    bq_d = nc.dram_tensor("bq", [D], FP32, kind="ExternalInput")
    wk_d = nc.dram_tensor("Wk", [DIN, D], BF16, kind="ExternalInput")
    bk_d = nc.dram_tensor("bk", [D], FP32, kind="ExternalInput")
    wv_d = nc.dram_tensor("Wv", [DIN, D], BF16, kind="ExternalInput")
    bv_d = nc.dram_tensor("bv", [D], BF16, kind="ExternalInput")
    out_d = nc.dram_tensor("out", [S, D], FP32, kind="ExternalOutput")

    with tile.TileContext(nc) as tc:
        with (
            tc.tile_pool(name="const", bufs=1) as constp,
            tc.tile_pool(name="big", bufs=1) as bigp,
        ):
            ident = constp.tile([P, P], BF16)
            make_identity(nc, ident[:])
            ones_row = constp.tile([1, P], BF16)
            nc.gpsimd.memset(ones_row[:], 1.0)

            # Weights: [128, 4, 256] with [:, c, :] = W[c*128:(c+1)*128, :]
            # (constants go on the GpSimd DMA queue so the bulk x loads on
            # the Sync queue aren't stuck behind their many descriptors)
            # bv first: the bv broadcast matmul is early in PE program
            # order, so its DMA must not queue behind the weight loads
            bv_row = constp.tile([1, D], BF16)
            nc.gpsimd.dma_start(bv_row[:], bv_d.rearrange("(o d) -> o d", o=1))
            wq_sb = constp.tile([P, DC, D], BF16)
            wk_sb = constp.tile([P, DC, D], BF16)
            wv_sb = constp.tile([P, DC, D], BF16)
            nc.gpsimd.dma_start(wq_sb[:], wq_d.rearrange("(c p) d -> p c d", p=P))
            nc.gpsimd.dma_start(wk_sb[:], wk_d.rearrange("(c p) d -> p c d", p=P))
            nc.gpsimd.dma_start(wv_sb[:], wv_d.rearrange("(c p) d -> p c d", p=P))
            # Per-partition bias layout for QT/KT: [:, c] = b[c*128:(c+1)*128]
            bqT = constp.tile([P, ECH], FP32)
            bkT = constp.tile([P, ECH], FP32)
            nc.gpsimd.dma_start(bqT[:], bq_d.rearrange("(c p) -> p c", p=P))
            nc.gpsimd.dma_start(bkT[:], bk_d.rearrange("(c p) -> p c", p=P))

            qt = bigp.tile([P, ECH, S], BF16)   # QT: [e-chunk part, ec, s]
            kt = bigp.tile([P, ECH, S], BF16)
            vext = bigp.tile([P, S_TILES, VE_PAD], BF16)  # V + ones col
            nc.gpsimd.memset(vext[:, :, D : D + 1], 1.0)

            # bv broadcast tile (filled below, after the first transposes,
            # so its rank-1 matmul does not gate the PE pipeline start)
            bv_bc = constp.tile([P, D], BF16)

            # ---- Phase 1+2 fused: transpose x -> xT, with each s-block's
            # projections (Q/K/V) emitted right after its 4 transposes.
            # Interleaving the tiny N=128 transpose matmuls with N=512
            # projection streams lets the PE's LDWEIGHTS pull-ahead hide
            # the transpose weight loads under the projection streams. ----
            with tc.tile_pool(name="xTpool", bufs=1) as xtp:
                xt = xtp.tile([P, DC, S], BF16)  # xT: [din-chunk part, dc, s]
                with (
                    tc.tile_pool(name="xload", bufs=8) as xlp,
                    tc.tile_pool(name="tps", bufs=3, space="PSUM") as tpsp,
                    tc.tile_pool(name="pjq", bufs=3, space="PSUM") as pjq,
                    tc.tile_pool(name="pjv", bufs=2, space="PSUM") as pjv,
                ):
                    xtiles, psts = {}, {}

                    def emit_t_dma(st):
                        xtile = xlp.tile([P, DIN], BF16, name="xtile")
                        nc.sync.dma_start(
                            xtile[:], x_d[st * P : (st + 1) * P, :]
                        )
                        xtiles[st] = xtile

                    def emit_t_mm(st, c):
                        # one transposed [128,128] chunk; 4 chunks fill one
                        # PSUM bank, then a single strided cast to SBUF
                        if c == 0:
                            psts[st] = tpsp.tile([P, DIN], FP32, name="pst")
                        nc.tensor.matmul(
                            psts[st][:, c * P : (c + 1) * P],
                            xtiles[st][:, c * P : (c + 1) * P],
                            ident[:],
                            start=True,
                            stop=True,
                        )
                        if c == DC - 1:
                            src = psts.pop(st)[:].rearrange(
                                "p (c f) -> p c f", c=DC
                            )
                            dstv = xt[:, :, st * P : (st + 1) * P]
                            if st % 2 == 0:
                                nc.vector.tensor_copy(dstv, src)
                            else:
                                nc.scalar.copy(dstv, src)
                            xtiles.pop(st)

                    # group 0 has nothing to hide under (pure DMA wait)
                    for st in range(4):
                        emit_t_dma(st)
                    for st in range(4):
                        for c in range(DC):
                            emit_t_mm(st, c)

                    # bv broadcast to all partitions (one rank-1 matmul +
                    # copy, borrowing a pjv bank); V tiles then get the
                    # bias via a DVE add on the PSUM->SBUF copy instead
                    # of a rank-1 matmul per s-tile.
                    psb = pjv.tile([P, D], FP32, name="psb", tag="psv")
                    nc.tensor.matmul(
                        psb[:], ones_row[:], bv_row[:], start=True, stop=True
                    )
                    nc.vector.tensor_copy(bv_bc[:], psb[:])

                    for sb in range(N_QB):
                        # next group's transposes: 16 tiny matmuls, paired
                        # 1:1 with this block's 16 N=512 Q/K matmuls so
                        # their weight loads hide under the streams
                        tmms = []
                        if sb + 1 < N_QB:
                            for st in range(4 * (sb + 1), 4 * (sb + 1) + 4):
                                emit_t_dma(st)
                                tmms += [(st, c) for c in range(DC)]
                        ti = 0
                        for w_sb, bT, dst in (
                            (wq_sb, bqT, qt),
                            (wk_sb, bkT, kt),
                        ):
                            for ec in range(ECH):
                                ps = pjq.tile([P, QB], FP32)
                                for dc in range(DC):
                                    nc.tensor.matmul(
                                        ps[:],
                                        w_sb[:, dc, ec * P : (ec + 1) * P],
                                        xt[:, dc, sb * QB : (sb + 1) * QB],
                                        start=(dc == 0),
                                        stop=(dc == DC - 1),
                                    )
                                    if ti < len(tmms):
                                        emit_t_mm(*tmms[ti])
                                        ti += 1
                                nc.scalar.activation(
                                    dst[:, ec, sb * QB : (sb + 1) * QB],
                                    ps[:],
                                    AF.Identity,
                                    bias=bT[:, ec : ec + 1],
                                )
                        for stv in range(sb * 4, sb * 4 + 4):
                            psv = pjv.tile([P, D], FP32)
                            for dc in range(DC):
                                nc.tensor.matmul(
                                    psv[:],
                                    xt[:, dc, stv * P : (stv + 1) * P],
                                    wv_sb[:, dc, :],
                                    start=(dc == 0),
                                    stop=(dc == DC - 1),
                                )
                            nc.vector.tensor_add(
                                vext[:, stv, 0:D], psv[:], bv_bc[:]
                            )

            # ---- Phase 3: attention (software-pipelined: scores run
            # LOOKAHEAD k-tiles ahead of attn@V so the PE never waits on
            # the ACT exp latency) ----
            LOOKAHEAD = 3
            NSTEPS = N_QB * S_TILES
            with (
                tc.tile_pool(name="ptp", bufs=5) as ptp,
                tc.tile_pool(name="accp", bufs=5, space="PSUM") as accp,
                tc.tile_pool(name="scp", bufs=3, space="PSUM") as scp,
                tc.tile_pool(name="outp", bufs=4) as outp,
                tc.tile_pool(name="nrmp", bufs=4) as nrmp,
            ):
                accs = {}
                ptts = {}
                # one flat loop over (q-block, k-tile) so the scores
                # lookahead also spans q-block boundaries
                for step in range(NSTEPS + LOOKAHEAD):
                    if step < NSTEPS:
                        qb, kt_i = divmod(step, S_TILES)
                        if kt_i == 0:
                            accs[qb] = [
                                accp.tile([P, VE], FP32, name="acc", tag="acc")
                                for _ in range(QB // P)
                            ]
                        pss = scp.tile([P, QB], FP32)
                        for ec in range(ECH):
                            nc.tensor.matmul(
                                pss[:],
                                kt[:, ec, kt_i * P : (kt_i + 1) * P],
                                qt[:, ec, qb * QB : (qb + 1) * QB],
                                start=(ec == 0),
                                stop=(ec == ECH - 1),
                            )
                        ptt = ptp.tile([P, QB], BF16)
                        nc.scalar.activation(
                            ptt[:], pss[:], AF.Exp, scale=SCALE
                        )
                        ptts[step] = ptt
                    av = step - LOOKAHEAD
                    if av >= 0:
                        qb2, kt2 = divmod(av, S_TILES)
                        pav = ptts.pop(av)
                        for j in range(QB // P):
                            nc.tensor.matmul(
                                accs[qb2][j][:],
                                pav[:, j * P : (j + 1) * P],
                                vext[:, kt2, 0:VE],
                                start=(kt2 == 0),
                                stop=(kt2 == S_TILES - 1),
                            )
                        if kt2 == S_TILES - 1:
                            for j in range(QB // P):
                                rc = nrmp.tile([P, 1], FP32)
                                nc.vector.reciprocal(
                                    rc[:], accs[qb2][j][:, D : D + 1]
                                )
                                ot = outp.tile([P, D], FP32)
                                nc.vector.tensor_scalar_mul(
                                    ot[:], accs[qb2][j][:, 0:D], rc[:]
                                )
                                row = (qb2 * (QB // P) + j) * P
                                nc.sync.dma_start(
                                    out_d[row : row + P, :], ot[:]
                                )
                            del accs[qb2]

    nc.compile()
    return nc


_NC_CACHE = []


def _get_nc():
    if not _NC_CACHE:
        _NC_CACHE.append(build_program())
    return _NC_CACHE[0]


def kernel(**inputs) -> np.ndarray:
    BF = ml_dtypes.bfloat16
    x = np.ascontiguousarray(np.asarray(inputs["x"]).astype(BF))
    w = {}
    for k in ("Wq", "Wk", "Wv", "bv"):
        w[k] = np.ascontiguousarray(np.asarray(inputs[k]).astype(BF))
    for k in ("bq", "bk"):
        w[k] = np.ascontiguousarray(np.asarray(inputs[k]).astype(np.float32))
    nc = _get_nc()
    in_maps = [{"x": x[b], **w} for b in range(B)]
    res = run_bass_kernel_spmd(nc, in_maps, list(range(N_CORES)))
    return np.stack([res.results[b]["out"] for b in range(B)], axis=0)
